# revision 38
# baseline (speedup 1.0000x reference)
"""CondConv2d (MoE routed conv) Trainium2 Bass kernel — v2.

Strategy (v2: 1D Winograd F(2,3) along W, bf16 matmuls)
-------------------------------------------------------
Data-parallel over batch B=32 across 8 NeuronCores (4 samples/core); the
expert bank + routing params are replicated.

The 3x3 conv is computed as a direct 3-tap conv along H but Winograd
F(2,3) along W, cutting PE work 1.5x (4 transform positions replace 6
tap-products per 2 output columns).  All matmul operands are bf16
(1 cycle/row on the PE, same rate as f32r, but half the SBUF/DMA and
2-4x DVE rate for the transforms).

Per core and sample:
  1. Host pre-pads x to 58x58 and de-interleaves W into even/odd planes
     (bf16).  Device DMAs [128, 2*58*29] per c-chunk.
  2. pooled = sum(x) via DVE reduce (padding zeros are harmless);
     routing MLP on PE/ACT/DVE; softmax-normalized routing weights are
     partition-broadcast by GPSIMD.
  3. Input transform (DVE, 4 tensor_tensor adds per c-chunk):
     V0=E[t]-E[t+1], V1=D[t]+E[t+1], V2=E[t+1]-D[t], V3=D[t]-D[t+1],
     giving V[c, xi, 58, 28] bf16.
  4. Expert bank is host-transformed to the Winograd domain
     [C, E, 3dh, 4xi, O] (the weight transform is linear, so
     mix-then-transform == transform-then-mix).  Device mixing is a
     scalar_tensor_tensor chain on DVE with the (normalized) routing
     scalars -> combined[c, (dh xi o)] bf16.
  5. Conv: psum M[xi][o, 14h, 28t] accumulates 6 matmuls (2 c-chunks x
     3 dh row-shifts), N=392 rows each.  192 matmuls/sample vs 252 for
     direct conv, at 392 vs 448 rows -> 75,264 vs 112,896 PE cycles.
  6. Output transform on PSUM evac (DVE): even=M0+M1+M2, odd=M1-M2-M3,
     staged bf16 per (o-chunk, parity) and DMA'd as separate even/odd
     planes; the host re-interleaves W and converts to f32.
"""

import numpy as np
import ml_dtypes
from contextlib import ExitStack

import concourse.bass as bass
import concourse.bacc as bacc
import concourse.mybir as mybir
import concourse.tile as tile
from concourse.bass_utils import run_bass_kernel_spmd

F32 = mybir.dt.float32
BF16 = mybir.dt.bfloat16
NPBF = ml_dtypes.bfloat16
AF = mybir.ActivationFunctionType
ALU = mybir.AluOpType
AX = mybir.AxisListType

# Problem shapes (hardcoded per contract).
B, C, H, W = 32, 256, 56, 56
E, O, K = 4, 256, 3
HID = 64
NCORES = 8
BL = B // NCORES          # samples per core
CCH = C // 128            # c partition chunks
OCH = O // 128            # o partition chunks
HP = H + 2                # padded rows
NXI = 4                   # Winograd F(2,3) positions
T = W // 2                # 28 Winograd column tiles
TP = T + 1                # 29 = de-interleaved plane width (with pad col)
HBLK = 14                 # output rows per psum tile
NRB = H // HBLK           # 4 row blocks
NFREE = HBLK * T          # 392 = matmul free size
SEG = 3 * NXI * O         # 3072 = combined-weight cols per c-chunk

_CACHE = {}


def _build_program():
    nc = bacc.Bacc("TRN2", target_bir_lowering=False, debug=False)

    x_d = nc.dram_tensor("v_in", [BL, C, NXI, HP, T], BF16,
                         kind="ExternalInput").ap()
    ex_d = nc.dram_tensor("experts_w", [C, E, OCH, 3 * NXI * 128], BF16,
                          kind="ExternalInput").ap()
    # bf16 routing params: [:,0:64]=rw1t cc0, [:,64:128]=rw1t cc1,
    # [0:64,128:132]=rw2t
    rpb_d = nc.dram_tensor("rparams_bf", [128, 2 * HID + E], BF16,
                           kind="ExternalInput").ap()
    # f32 routing params: [0:64,0]=rb1, [0,1:5]=rb2
    rpf_d = nc.dram_tensor("rparams_f", [128, 1 + E], F32,
                           kind="ExternalInput").ap()
    out_d = nc.dram_tensor("out_eo", [BL, 2, O, H, T], BF16,
                           kind="ExternalOutput").ap()

    with tile.TileContext(nc) as tc, ExitStack() as ctx:
        const_pool = ctx.enter_context(tc.tile_pool(name="const", bufs=1))
        v_pool = ctx.enter_context(tc.tile_pool(name="vwin", bufs=2 * CCH))
        cmb_pool = ctx.enter_context(tc.tile_pool(name="comb", bufs=2 * CCH))
        mscr_pool = ctx.enter_context(tc.tile_pool(name="mscr", bufs=4))
        ct_pool = ctx.enter_context(tc.tile_pool(name="ct", bufs=2))
        escr_pool = ctx.enter_context(tc.tile_pool(name="escr", bufs=4))
        ostg_pool = ctx.enter_context(tc.tile_pool(name="ostg", bufs=4))
        small_pool = ctx.enter_context(tc.tile_pool(name="small", bufs=2))
        cpsum_pool = ctx.enter_context(
            tc.tile_pool(name="cpsum", bufs=7, space="PSUM"))
        mpsum_pool = ctx.enter_context(
            tc.tile_pool(name="mpsum", bufs=1, space="PSUM"))

        # ---- constants / parameters (preload once) ----
        rpb_t = const_pool.tile([128, 2 * HID + E], BF16, name="rpb")
        nc.sync.dma_start(rpb_t[:], rpb_d[:])
        rpf_t = const_pool.tile([128, 1 + E], F32, name="rpf")
        nc.sync.dma_start(rpf_t[:], rpf_d[:])
        rw1t_t = [rpb_t[:, 0:HID], rpb_t[:, HID:2 * HID]]
        rw2t_t = rpb_t[0:HID, 2 * HID:2 * HID + E]
        rb1_t = rpf_t[0:HID, 0:1]
        rb2_t = rpf_t[0:1, 1:1 + E]

        slabs = []   # [cc][e] -> [128, SEG] bf16 winograd-domain experts

        # slabs[cc][e][oc]: [128, HSEG] winograd-domain expert oc-halves
        HSEG = SEG // OCH
        for cc in range(CCH):
            slabs.append([[const_pool.tile([128, HSEG], BF16,
                                           name=f"slab{cc}e{e}o{oc}")
                           for oc in range(OCH)] for e in range(E)])

        def emit_slab_loads(oc):
            # one oc-half at a time, e-major.  The DMA pool drains roughly
            # in issue order, so these go on sync AFTER sample 0's V loads:
            # V lands first (pooling/routing path), then oc0 expert bytes
            # (sample 0's first mixing), then oc1.
            for e in range(E):
                for cc in range(CCH):
                    nc.sync.dma_start(
                        slabs[cc][e][oc][:],
                        ex_d[cc * 128:(cc + 1) * 128, e, oc])

        # per-sample state
        vt = {}       # (b, cc) -> V tile [128, NXI, HP, T] bf16
        comb = {}     # (b, cc, oc) -> combined weights [128, HSEG] bf16
        pooled_t = {}  # (b, cc) -> [128, 1] bf16 sum over h*w

        def emit_loads(b):
            # DMA only (one per Winograd plane, spread across queues); the
            # engine-side pooling runs later in emit_routing so the strict
            # ACT FIFO never stalls on these DMAs ahead of evac copies.
            for cc in range(CCH):
                v = v_pool.tile([128, NXI, HP, T], BF16, tag="v",
                                name=f"v{b}_{cc}")
                vflat = v.rearrange("p x h t -> p (x h t)")
                xflat = x_d[b, cc * 128:(cc + 1) * 128].rearrange(
                    "p x h t -> p (x h t)")
                if b == 0:
                    # split so pooling (plane 1) can start at half arrival
                    half = NXI * HP * T // 2
                    nc.sync.dma_start(vflat[:, 0:half], xflat[:, 0:half])
                    nc.sync.dma_start(vflat[:, half:], xflat[:, half:])
                else:
                    nc.sync.dma_start(vflat, xflat)
                vt[(b, cc)] = v

        def emit_pooling(b):
            # pooled(x) == sum(V1): V1 = D[t] + E[t+1] covers every unpadded
            # column of x exactly once (E[0]/D[28] are the zero pad cols).
            # ACT copy-with-accum keeps it off the DVE.
            for cc in range(CCH):
                v = vt[(b, cc)]
                pf = small_pool.tile([128, 1], F32, tag="poolf", bufs=8,
                                     name=f"poolf{b}_{cc}")
                junk = escr_pool.tile([128, HP * T], BF16, tag="junk", bufs=2,
                                      name=f"junk{b}_{cc}")
                nc.scalar.activation(junk[:],
                                     v[:, 1].rearrange("p h t -> p (h t)"),
                                     AF.Copy, accum_out=pf[:])
                pb = small_pool.tile([128, 1], BF16, tag="poolb", bufs=8,
                                     name=f"poolb{b}_{cc}")
                nc.scalar.copy(pb[:], pf[:])
                pooled_t[(b, cc)] = pb

        def emit_routing(b):
            emit_pooling(b)
            mps = mpsum_pool.tile([128, 16], F32, tag="mps", name=f"mps{b}")
            for cc in range(CCH):
                nc.tensor.matmul(mps[0:HID, 0:1], rw1t_t[cc],
                                 pooled_t[(b, cc)][:],
                                 start=(cc == 0), stop=(cc == CCH - 1))
            h_sb = small_pool.tile([HID, 1], BF16, tag="h", name=f"h{b}")
            nc.scalar.activation(h_sb[:], mps[0:HID, 0:1], AF.Relu,
                                 bias=rb1_t[:])
            nc.tensor.matmul(mps[0:1, 4:4 + E], h_sb[:], rw2t_t[:],
                             start=True, stop=True)
            ze = small_pool.tile([1, E], F32, tag="ze", name=f"ze{b}")
            nc.vector.tensor_add(ze[:], mps[0:1, 4:4 + E], rb2_t[:])
            es = small_pool.tile([1, E], F32, tag="es", name=f"es{b}")
            nc.scalar.activation(es[:], ze[:], AF.Exp)
            # mix with UNNORMALIZED exp weights; 1/sum is applied later as
            # the scale of the ACT psum-evacuation copies (keeps the softmax
            # normalization off the mixing critical path)
            rbc = small_pool.tile([128, E], F32, tag="rbc", name=f"rbc{b}")
            nc.gpsimd.partition_broadcast(rbc[:], es[0:1, 0:E])
            ssum = small_pool.tile([1, 1], F32, tag="ssum", name=f"ss{b}")
            nc.vector.reduce_sum(out=ssum[:], in_=es[:], axis=AX.X)
            rec = small_pool.tile([1, 1], F32, tag="rec", name=f"rec{b}")
            nc.vector.reciprocal(rec[:], ssum[:])
            rvb = small_pool.tile([128, 1], F32, tag="rvb", name=f"rv{b}")
            nc.gpsimd.partition_broadcast(rvb[:], rec[0:1, 0:1])
            return rbc, rvb

        def emit_mixing(b, rbc, oc):
            # tensor_scalar_mul hits the DVE bf16 fast path (~3.5x faster
            # than scalar_tensor_tensor); tree-add the 4 scaled slabs.
            for cc in range(CCH):
                slab = slabs[cc]
                m = [mscr_pool.tile([128, HSEG], BF16, tag="mscr",
                                    name=f"ms{b}_{cc}_{oc}_{e}")
                     for e in range(E)]
                for e in range(E):
                    nc.vector.tensor_scalar_mul(m[e][:], slab[e][oc][:],
                                                rbc[:, e:e + 1])
                nc.vector.tensor_add(m[2][:], m[2][:], m[3][:])
                nc.vector.tensor_add(m[0][:], m[0][:], m[1][:])
                cmb = cmb_pool.tile([128, HSEG], BF16, tag="cmb",
                                    name=f"cmb{b}_{cc}_{oc}")
                nc.vector.tensor_add(cmb[:], m[0][:], m[2][:])
                comb[(b, cc, oc)] = cmb

        def emit_conv_ochunk(b, oc, rvb):
            oeven = ostg_pool.tile([128, H * T], BF16, tag="ostg",
                                   name=f"oe{b}_{oc}")
            oodd = ostg_pool.tile([128, H * T], BF16, tag="ostg",
                                  name=f"oo{b}_{oc}")
            ct = None
            for rb in range(NRB):
                # ACT evacuates psum into a bf16 staging tile (one psum read
                # per op); DVE then runs the output transform as few, fat,
                # all-SBUF bf16 ops over rb-pairs.
                if rb % 2 == 0:
                    ct = ct_pool.tile([128, NXI, 2 * NFREE], BF16, tag="ct",
                                      name=f"ct{b}_{oc}_{rb // 2}")
                pt = [cpsum_pool.tile([128, NFREE], F32, tag="cps",
                                      name=f"cp{b}_{oc}_{rb}_{xi}")
                      for xi in range(NXI)]
                i = 0
                for cc in range(CCH):
                    cmb = comb[(b, cc, oc)]
                    v = vt[(b, cc)]
                    for dh in range(3):
                        for xi in range(NXI):
                            lo = (dh * NXI + xi) * 128
                            rhs = v[:, xi, rb * HBLK + dh: rb * HBLK + dh + HBLK,
                                    0:T]
                            nc.tensor.matmul(pt[xi][:], cmb[:, lo:lo + 128],
                                             rhs, start=(i < NXI),
                                             stop=(i >= (2 * 3 - 1) * NXI))
                        i += NXI
                hs = slice((rb % 2) * NFREE, (rb % 2 + 1) * NFREE)
                for xi in range(NXI):
                    # evac copy applies the deferred softmax 1/sum scale
                    nc.scalar.mul(ct[:, xi, hs], pt[xi][:], rvb[:, 0:1])
                if rb % 2 == 1:
                    sl = slice((rb - 1) * NFREE, (rb + 1) * NFREE)
                    t1 = escr_pool.tile([128, 2 * NFREE], BF16, tag="escr",
                                        name=f"t1{b}_{oc}_{rb}")
                    nc.vector.tensor_add(t1[:], ct[:, 0], ct[:, 1])
                    nc.vector.tensor_add(oeven[:, sl], t1[:], ct[:, 2])
                    t2 = escr_pool.tile([128, 2 * NFREE], BF16, tag="escr",
                                        name=f"t2{b}_{oc}_{rb}")
                    nc.vector.tensor_sub(t2[:], ct[:, 1], ct[:, 2])
                    nc.vector.tensor_sub(oodd[:, sl], t2[:], ct[:, 3])
                    if b == BL - 1 and oc == OCH - 1:
                        # stream the final chunk per rb-pair to cut the tail
                        ov = out_d[b, :, oc * 128:(oc + 1) * 128,
                                   (rb - 1) * HBLK:(rb + 1) * HBLK].rearrange(
                            "a p h t -> a p (h t)")
                        nc.gpsimd.dma_start(ov[0], oeven[:, sl])
                        nc.gpsimd.dma_start(ov[1], oodd[:, sl])
            if b == BL - 1 and oc == OCH - 1:
                return
            # out DMAs issue from the idle GPSIMD queue (sync stays free
            # for input loads)
            ov = out_d[b, :, oc * 128:(oc + 1) * 128].rearrange(
                "a p h t -> a p (h t)")
            nc.gpsimd.dma_start(ov[0], oeven[:])
            nc.gpsimd.dma_start(ov[1], oodd[:])

        # ---- emission: software-pipelined across samples ----
        emit_loads(0)
        emit_slab_loads(0)
        emit_slab_loads(1)
        route = {0: emit_routing(0)}
        emit_mixing(0, route[0][0], 0)
        for b in range(BL):
            emit_mixing(b, route[b][0], 1)
            emit_conv_ochunk(b, 0, route[b][1])
            if b + 1 < BL:
                emit_loads(b + 1)
                route[b + 1] = emit_routing(b + 1)
                emit_mixing(b + 1, route[b + 1][0], 0)
            emit_conv_ochunk(b, 1, route[b][1])

    nc.compile()
    return nc


def _prep_inputs(x, experts, rw1, rb1, rw2, rb2):
    x = np.asarray(x, dtype=np.float32)
    xp = np.pad(x, ((0, 0), (0, 0), (1, 1), (1, 1)))
    # Winograd F(2,3) input transform along W (linear data prep):
    # [B, C, 4xi, 58, 28]
    ev = xp[:, :, :, 0::2]
    dv = xp[:, :, :, 1::2]
    v_in = np.ascontiguousarray(np.stack([
        ev[:, :, :, 0:T] - ev[:, :, :, 1:TP],
        dv[:, :, :, 0:T] + ev[:, :, :, 1:TP],
        ev[:, :, :, 1:TP] - dv[:, :, :, 0:T],
        dv[:, :, :, 0:T] - dv[:, :, :, 1:TP],
    ], axis=2)).astype(NPBF)
    g = np.asarray(experts, dtype=np.float32)        # [E, O, C, 3, 3]
    w0, w1, w2 = g[..., 0], g[..., 1], g[..., 2]     # [E, O, C, 3dh]
    wt = np.stack([w0, (w0 + w1 + w2) * 0.5, (w0 - w1 + w2) * 0.5, w2],
                  axis=-1)                           # [E, O, C, 3dh, 4xi]
    # [C, E, 3, 4, O] -> oc-major halves [C, E, 2oc, (3 4 128)]
    ex_t = np.transpose(wt, (2, 0, 3, 4, 1)).reshape(C, E, 3 * NXI, OCH, 128)
    ex_t = np.ascontiguousarray(
        np.transpose(ex_t, (0, 1, 3, 2, 4))).astype(NPBF)
    ex_t = ex_t.reshape(C, E, OCH, 3 * NXI * 128)
    rw1t = (np.asarray(rw1, dtype=np.float32) / float(H * W)).T  # [C, HID]
    rw2t = np.asarray(rw2, dtype=np.float32).T                   # [HID, E]
    rpb = np.zeros((128, 2 * HID + E), np.float32)
    rpb[:, 0:HID] = rw1t[0:128]
    rpb[:, HID:2 * HID] = rw1t[128:256]
    rpb[0:HID, 2 * HID:2 * HID + E] = rw2t
    rpb = rpb.astype(NPBF)
    rpf = np.zeros((128, 1 + E), np.float32)
    rpf[0:HID, 0] = np.asarray(rb1, dtype=np.float32)
    rpf[0, 1:1 + E] = np.asarray(rb2, dtype=np.float32)
    in_maps = []
    for i in range(NCORES):
        in_maps.append({
            "v_in": np.ascontiguousarray(v_in[i * BL:(i + 1) * BL]),
            "experts_w": ex_t,
            "rparams_bf": rpb,
            "rparams_f": rpf,
        })
    return in_maps


def run(inputs, trace=False, **trace_kwargs):
    """Build (cached), run on 8 cores, return (full_out, BassKernelResults)."""
    trace_kwargs.pop("use_f32r", None)
    if "prog" not in _CACHE:
        _CACHE["prog"] = _build_program()
    nc = _CACHE["prog"]
    in_maps = _prep_inputs(**inputs)
    res = run_bass_kernel_spmd(nc, in_maps, list(range(NCORES)),
                               trace=trace, **trace_kwargs)
    out = np.empty((B, O, H, W), dtype=np.float32)
    for i in range(NCORES):
        oeo = np.asarray(res.results[i]["out_eo"], dtype=np.float32)
        out[i * BL:(i + 1) * BL, :, :, 0::2] = oeo[:, 0]
        out[i * BL:(i + 1) * BL, :, :, 1::2] = oeo[:, 1]
    return out, res


def kernel(x, experts, rw1, rb1, rw2, rb2):
    out, _ = run(dict(x=x, experts=experts, rw1=rw1, rb1=rb1, rw2=rw2,
                      rb2=rb2))
    return out


# revision 45
# speedup vs baseline: 1.1782x; 1.1782x over previous
"""CondConv2d (MoE routed conv) Trainium2 Bass kernel — v2.

Strategy (v2: 1D Winograd F(2,3) along W, bf16 matmuls)
-------------------------------------------------------
Data-parallel over batch B=32 across 8 NeuronCores (4 samples/core); the
expert bank + routing params are replicated.

The 3x3 conv is computed as a direct 3-tap conv along H but Winograd
F(2,3) along W, cutting PE work 1.5x (4 transform positions replace 6
tap-products per 2 output columns).  All matmul operands are bf16
(1 cycle/row on the PE, same rate as f32r, but half the SBUF/DMA and
2-4x DVE rate for the transforms).

Per core and sample:
  1. Host pre-pads x to 58x58 and de-interleaves W into even/odd planes
     (bf16).  Device DMAs [128, 2*58*29] per c-chunk.
  2. pooled = sum(x) via DVE reduce (padding zeros are harmless);
     routing MLP on PE/ACT/DVE; softmax-normalized routing weights are
     partition-broadcast by GPSIMD.
  3. Input transform (DVE, 4 tensor_tensor adds per c-chunk):
     V0=E[t]-E[t+1], V1=D[t]+E[t+1], V2=E[t+1]-D[t], V3=D[t]-D[t+1],
     giving V[c, xi, 58, 28] bf16.
  4. Expert bank is host-transformed to the Winograd domain
     [C, E, 3dh, 4xi, O] (the weight transform is linear, so
     mix-then-transform == transform-then-mix).  Device mixing is a
     scalar_tensor_tensor chain on DVE with the (normalized) routing
     scalars -> combined[c, (dh xi o)] bf16.
  5. Conv: psum M[xi][o, 14h, 28t] accumulates 6 matmuls (2 c-chunks x
     3 dh row-shifts), N=392 rows each.  192 matmuls/sample vs 252 for
     direct conv, at 392 vs 448 rows -> 75,264 vs 112,896 PE cycles.
  6. Output transform on PSUM evac (DVE): even=M0+M1+M2, odd=M1-M2-M3,
     staged bf16 per (o-chunk, parity) and DMA'd as separate even/odd
     planes; the host re-interleaves W and converts to f32.
"""

import numpy as np
import ml_dtypes
from contextlib import ExitStack

import concourse.bass as bass
import concourse.bacc as bacc
import concourse.mybir as mybir
import concourse.tile as tile
from concourse.bass_utils import run_bass_kernel_spmd

F32 = mybir.dt.float32
BF16 = mybir.dt.bfloat16
NPBF = ml_dtypes.bfloat16
AF = mybir.ActivationFunctionType
ALU = mybir.AluOpType
AX = mybir.AxisListType

# Problem shapes (hardcoded per contract).
B, C, H, W = 32, 256, 56, 56
E, O, K = 4, 256, 3
HID = 64
NCORES = 8
BL = B // NCORES          # samples per core
CCH = C // 128            # c partition chunks
OCH = O // 128            # o partition chunks
HP = H + 2                # padded rows
NXI = 4                   # Winograd F(2,3) positions
T = W // 2                # 28 Winograd column tiles
TP = T + 1                # 29 = de-interleaved plane width (with pad col)
HBLK = 14                 # output rows per psum tile
NRB = H // HBLK           # 4 row blocks
NFREE = HBLK * T          # 392 = matmul free size
SEG = 3 * NXI * O         # 3072 = combined-weight cols per c-chunk

_CACHE = {}


def _build_program():
    nc = bacc.Bacc("TRN2", target_bir_lowering=False, debug=False)

    x_d = nc.dram_tensor("v_in", [BL, C, NXI, HP, T], BF16,
                         kind="ExternalInput").ap()
    ex_d = nc.dram_tensor("experts_w", [C, E, OCH, 3 * NXI * 128], BF16,
                          kind="ExternalInput").ap()
    # bf16 routing params: [:,0:64]=rw1t cc0, [:,64:128]=rw1t cc1,
    # [0:64,128:132]=rw2t
    rpb_d = nc.dram_tensor("rparams_bf", [128, 2 * HID + E], BF16,
                           kind="ExternalInput").ap()
    # f32 routing params: [0:64,0]=rb1, [0,1:5]=rb2
    rpf_d = nc.dram_tensor("rparams_f", [128, 1 + E], F32,
                           kind="ExternalInput").ap()
    out_d = nc.dram_tensor("out_eo", [BL, 2, O, H, T], BF16,
                           kind="ExternalOutput").ap()

    with tile.TileContext(nc) as tc, ExitStack() as ctx:
        const_pool = ctx.enter_context(tc.tile_pool(name="const", bufs=1))
        v_pool = ctx.enter_context(tc.tile_pool(name="vwin", bufs=2 * CCH))
        cmb_pool = ctx.enter_context(tc.tile_pool(name="comb", bufs=2 * CCH))
        mscr_pool = ctx.enter_context(tc.tile_pool(name="mscr", bufs=4))
        ct_pool = ctx.enter_context(tc.tile_pool(name="ct", bufs=2))
        escr_pool = ctx.enter_context(tc.tile_pool(name="escr", bufs=4))
        ostg_pool = ctx.enter_context(tc.tile_pool(name="ostg", bufs=4))
        small_pool = ctx.enter_context(tc.tile_pool(name="small", bufs=2))
        cpsum_pool = ctx.enter_context(
            tc.tile_pool(name="cpsum", bufs=7, space="PSUM"))
        mpsum_pool = ctx.enter_context(
            tc.tile_pool(name="mpsum", bufs=1, space="PSUM"))

        # ---- constants / parameters (preload once) ----
        rpb_t = const_pool.tile([128, 2 * HID + E], BF16, name="rpb")
        nc.sync.dma_start(rpb_t[:], rpb_d[:])
        rpf_t = const_pool.tile([128, 1 + E], F32, name="rpf")
        nc.sync.dma_start(rpf_t[:], rpf_d[:])
        rw1t_t = [rpb_t[:, 0:HID], rpb_t[:, HID:2 * HID]]
        rw2t_t = rpb_t[0:HID, 2 * HID:2 * HID + E]
        rb1_t = rpf_t[0:HID, 0:1]
        rb2_t = rpf_t[0:1, 1:1 + E]

        slabs = []   # [cc][e] -> [128, SEG] bf16 winograd-domain experts

        # slabs[cc][e][oc]: [128, HSEG] winograd-domain expert oc-halves
        HSEG = SEG // OCH
        for cc in range(CCH):
            slabs.append([[const_pool.tile([128, HSEG], BF16,
                                           name=f"slab{cc}e{e}o{oc}")
                           for oc in range(OCH)] for e in range(E)])

        def emit_slab_loads(oc):
            # one oc-half at a time, e-major.  The DMA pool drains roughly
            # in issue order, so these go on sync AFTER sample 0's V loads:
            # V lands first (pooling/routing path), then oc0 expert bytes
            # (sample 0's first mixing), then oc1.
            for e in range(E):
                for cc in range(CCH):
                    nc.sync.dma_start(
                        slabs[cc][e][oc][:],
                        ex_d[cc * 128:(cc + 1) * 128, e, oc])

        # per-sample state
        vt = {}       # (b, cc) -> V tile [128, NXI, HP, T] bf16
        comb = {}     # (b, cc, oc) -> combined weights [128, HSEG] bf16
        pooled_t = {}  # (b, cc) -> [128, 1] bf16 sum over h*w

        def emit_loads(b):
            # DMA only (one per Winograd plane, spread across queues); the
            # engine-side pooling runs later in emit_routing so the strict
            # ACT FIFO never stalls on these DMAs ahead of evac copies.
            for cc in range(CCH):
                v = v_pool.tile([128, NXI, HP, T], BF16, tag="v",
                                name=f"v{b}_{cc}")
                nc.sync.dma_start(
                    v.rearrange("p x h t -> p (x h t)"),
                    x_d[b, cc * 128:(cc + 1) * 128].rearrange(
                        "p x h t -> p (x h t)"))
                vt[(b, cc)] = v

        def emit_loads0_half(half):
            # sample 0's V in two waves: planes 0-1 (pooling needs plane 1)
            # land before the oc0 expert bytes, planes 2-3 right after.
            hl = NXI * HP * T // 2
            sl = slice(half * hl, (half + 1) * hl)
            for cc in range(CCH):
                if half == 0:
                    vt[(0, cc)] = v_pool.tile([128, NXI, HP, T], BF16,
                                              tag="v", name=f"v0_{cc}")
                v = vt[(0, cc)]
                nc.sync.dma_start(
                    v.rearrange("p x h t -> p (x h t)")[:, sl],
                    x_d[0, cc * 128:(cc + 1) * 128].rearrange(
                        "p x h t -> p (x h t)")[:, sl])

        def emit_pooling(b):
            # pooled(x) == sum(V1): V1 = D[t] + E[t+1] covers every unpadded
            # column of x exactly once (E[0]/D[28] are the zero pad cols).
            # ACT copy-with-accum keeps it off the DVE.
            for cc in range(CCH):
                v = vt[(b, cc)]
                pf = small_pool.tile([128, 1], F32, tag="poolf", bufs=8,
                                     name=f"poolf{b}_{cc}")
                junk = escr_pool.tile([128, HP * T], BF16, tag="junk", bufs=2,
                                      name=f"junk{b}_{cc}")
                nc.scalar.activation(junk[:],
                                     v[:, 1].rearrange("p h t -> p (h t)"),
                                     AF.Copy, accum_out=pf[:])
                pb = small_pool.tile([128, 1], BF16, tag="poolb", bufs=8,
                                     name=f"poolb{b}_{cc}")
                nc.scalar.copy(pb[:], pf[:])
                pooled_t[(b, cc)] = pb

        def emit_routing(b):
            emit_pooling(b)
            mps = mpsum_pool.tile([128, 16], F32, tag="mps", name=f"mps{b}")
            for cc in range(CCH):
                nc.tensor.matmul(mps[0:HID, 0:1], rw1t_t[cc],
                                 pooled_t[(b, cc)][:],
                                 start=(cc == 0), stop=(cc == CCH - 1))
            h_sb = small_pool.tile([HID, 1], BF16, tag="h", name=f"h{b}")
            nc.scalar.activation(h_sb[:], mps[0:HID, 0:1], AF.Relu,
                                 bias=rb1_t[:])
            nc.tensor.matmul(mps[0:1, 4:4 + E], h_sb[:], rw2t_t[:],
                             start=True, stop=True)
            ze = small_pool.tile([1, E], F32, tag="ze", name=f"ze{b}")
            nc.vector.tensor_add(ze[:], mps[0:1, 4:4 + E], rb2_t[:])
            es = small_pool.tile([1, E], F32, tag="es", name=f"es{b}")
            nc.scalar.activation(es[:], ze[:], AF.Exp)
            ssum = small_pool.tile([1, 1], F32, tag="ssum", name=f"ss{b}")
            nc.vector.reduce_sum(out=ssum[:], in_=es[:], axis=AX.X)
            rec = small_pool.tile([1, 1], F32, tag="rec", name=f"rec{b}")
            nc.vector.reciprocal(rec[:], ssum[:])
            esn = small_pool.tile([1, E], F32, tag="esn", name=f"esn{b}")
            nc.vector.tensor_scalar_mul(esn[:], es[:], rec[0:1, 0:1])
            rbc = small_pool.tile([128, E], F32, tag="rbc", name=f"rbc{b}")
            nc.gpsimd.partition_broadcast(rbc[:], esn[0:1, 0:E])
            return rbc

        def emit_mixing(b, rbc, oc):
            # tensor_scalar_mul hits the DVE bf16 fast path (~3.5x faster
            # than scalar_tensor_tensor); tree-add the 4 scaled slabs.
            for cc in range(CCH):
                slab = slabs[cc]
                m = [mscr_pool.tile([128, HSEG], BF16, tag="mscr",
                                    name=f"ms{b}_{cc}_{oc}_{e}")
                     for e in range(E)]
                # chained adds so each step can run as soon as its slab's
                # DMA lands (e-major arrival order)
                nc.vector.tensor_scalar_mul(m[0][:], slab[0][oc][:],
                                            rbc[:, 0:1])
                nc.vector.tensor_scalar_mul(m[1][:], slab[1][oc][:],
                                            rbc[:, 1:2])
                nc.vector.tensor_add(m[0][:], m[0][:], m[1][:])
                nc.vector.tensor_scalar_mul(m[2][:], slab[2][oc][:],
                                            rbc[:, 2:3])
                nc.vector.tensor_add(m[0][:], m[0][:], m[2][:])
                nc.vector.tensor_scalar_mul(m[3][:], slab[3][oc][:],
                                            rbc[:, 3:4])
                cmb = cmb_pool.tile([128, HSEG], BF16, tag="cmb",
                                    name=f"cmb{b}_{cc}_{oc}")
                nc.vector.tensor_add(cmb[:], m[0][:], m[3][:])
                comb[(b, cc, oc)] = cmb

        def emit_conv_ochunk(b, oc):
            oeven = ostg_pool.tile([128, H * T], BF16, tag="ostg",
                                   name=f"oe{b}_{oc}")
            oodd = ostg_pool.tile([128, H * T], BF16, tag="ostg",
                                  name=f"oo{b}_{oc}")
            ct = None
            for rb in range(NRB):
                # ACT evacuates psum into a bf16 staging tile (one psum read
                # per op); DVE then runs the output transform as few, fat,
                # all-SBUF bf16 ops over rb-pairs.
                if rb % 2 == 0:
                    ct = ct_pool.tile([128, NXI, 2 * NFREE], BF16, tag="ct",
                                      name=f"ct{b}_{oc}_{rb // 2}")
                pt = [cpsum_pool.tile([128, NFREE], F32, tag="cps",
                                      name=f"cp{b}_{oc}_{rb}_{xi}")
                      for xi in range(NXI)]
                i = 0
                for cc in range(CCH):
                    cmb = comb[(b, cc, oc)]
                    v = vt[(b, cc)]
                    for dh in range(3):
                        for xi in range(NXI):
                            lo = (dh * NXI + xi) * 128
                            rhs = v[:, xi, rb * HBLK + dh: rb * HBLK + dh + HBLK,
                                    0:T]
                            nc.tensor.matmul(pt[xi][:], cmb[:, lo:lo + 128],
                                             rhs, start=(i < NXI),
                                             stop=(i >= (2 * 3 - 1) * NXI))
                        i += NXI
                hs = slice((rb % 2) * NFREE, (rb % 2 + 1) * NFREE)
                for xi in range(NXI):
                    nc.scalar.copy(ct[:, xi, hs], pt[xi][:])
                if rb % 2 == 1:
                    sl = slice((rb - 1) * NFREE, (rb + 1) * NFREE)
                    t1 = escr_pool.tile([128, 2 * NFREE], BF16, tag="escr",
                                        name=f"t1{b}_{oc}_{rb}")
                    nc.vector.tensor_add(t1[:], ct[:, 0], ct[:, 1])
                    nc.vector.tensor_add(oeven[:, sl], t1[:], ct[:, 2])
                    t2 = escr_pool.tile([128, 2 * NFREE], BF16, tag="escr",
                                        name=f"t2{b}_{oc}_{rb}")
                    nc.vector.tensor_sub(t2[:], ct[:, 1], ct[:, 2])
                    nc.vector.tensor_sub(oodd[:, sl], t2[:], ct[:, 3])
            # out DMAs issue from the idle GPSIMD queue (sync stays free
            # for input loads)
            ov = out_d[b, :, oc * 128:(oc + 1) * 128].rearrange(
                "a p h t -> a p (h t)")
            nc.gpsimd.dma_start(ov[0], oeven[:])
            nc.gpsimd.dma_start(ov[1], oodd[:])

        # ---- emission: software-pipelined across samples ----
        emit_loads0_half(0)
        emit_slab_loads(0)
        emit_loads0_half(1)
        emit_slab_loads(1)
        route = {0: emit_routing(0)}
        emit_mixing(0, route[0], 0)
        for b in range(BL):
            emit_mixing(b, route[b], 1)
            emit_conv_ochunk(b, 0)
            if b + 1 < BL:
                emit_loads(b + 1)
                route[b + 1] = emit_routing(b + 1)
                emit_mixing(b + 1, route[b + 1], 0)
            emit_conv_ochunk(b, 1)

    nc.compile()
    return nc


def _prep_inputs(x, experts, rw1, rb1, rw2, rb2):
    x = np.asarray(x, dtype=np.float32)
    xp = np.pad(x, ((0, 0), (0, 0), (1, 1), (1, 1)))
    # Winograd F(2,3) input transform along W (linear data prep):
    # [B, C, 4xi, 58, 28]
    ev = xp[:, :, :, 0::2]
    dv = xp[:, :, :, 1::2]
    v_in = np.ascontiguousarray(np.stack([
        ev[:, :, :, 0:T] - ev[:, :, :, 1:TP],
        dv[:, :, :, 0:T] + ev[:, :, :, 1:TP],
        ev[:, :, :, 1:TP] - dv[:, :, :, 0:T],
        dv[:, :, :, 0:T] - dv[:, :, :, 1:TP],
    ], axis=2)).astype(NPBF)
    g = np.asarray(experts, dtype=np.float32)        # [E, O, C, 3, 3]
    w0, w1, w2 = g[..., 0], g[..., 1], g[..., 2]     # [E, O, C, 3dh]
    wt = np.stack([w0, (w0 + w1 + w2) * 0.5, (w0 - w1 + w2) * 0.5, w2],
                  axis=-1)                           # [E, O, C, 3dh, 4xi]
    # [C, E, 3, 4, O] -> oc-major halves [C, E, 2oc, (3 4 128)]
    ex_t = np.transpose(wt, (2, 0, 3, 4, 1)).reshape(C, E, 3 * NXI, OCH, 128)
    ex_t = np.ascontiguousarray(
        np.transpose(ex_t, (0, 1, 3, 2, 4))).astype(NPBF)
    ex_t = ex_t.reshape(C, E, OCH, 3 * NXI * 128)
    rw1t = (np.asarray(rw1, dtype=np.float32) / float(H * W)).T  # [C, HID]
    rw2t = np.asarray(rw2, dtype=np.float32).T                   # [HID, E]
    rpb = np.zeros((128, 2 * HID + E), np.float32)
    rpb[:, 0:HID] = rw1t[0:128]
    rpb[:, HID:2 * HID] = rw1t[128:256]
    rpb[0:HID, 2 * HID:2 * HID + E] = rw2t
    rpb = rpb.astype(NPBF)
    rpf = np.zeros((128, 1 + E), np.float32)
    rpf[0:HID, 0] = np.asarray(rb1, dtype=np.float32)
    rpf[0, 1:1 + E] = np.asarray(rb2, dtype=np.float32)
    in_maps = []
    for i in range(NCORES):
        in_maps.append({
            "v_in": np.ascontiguousarray(v_in[i * BL:(i + 1) * BL]),
            "experts_w": ex_t,
            "rparams_bf": rpb,
            "rparams_f": rpf,
        })
    return in_maps


def run(inputs, trace=False, **trace_kwargs):
    """Build (cached), run on 8 cores, return (full_out, BassKernelResults)."""
    trace_kwargs.pop("use_f32r", None)
    if "prog" not in _CACHE:
        _CACHE["prog"] = _build_program()
    nc = _CACHE["prog"]
    in_maps = _prep_inputs(**inputs)
    res = run_bass_kernel_spmd(nc, in_maps, list(range(NCORES)),
                               trace=trace, **trace_kwargs)
    out = np.empty((B, O, H, W), dtype=np.float32)
    for i in range(NCORES):
        oeo = np.asarray(res.results[i]["out_eo"], dtype=np.float32)
        out[i * BL:(i + 1) * BL, :, :, 0::2] = oeo[:, 0]
        out[i * BL:(i + 1) * BL, :, :, 1::2] = oeo[:, 1]
    return out, res


def kernel(x, experts, rw1, rb1, rw2, rb2):
    out, _ = run(dict(x=x, experts=experts, rw1=rw1, rb1=rb1, rw2=rw2,
                      rb2=rb2))
    return out


# revision 47
# speedup vs baseline: 1.1991x; 1.0178x over previous
"""CondConv2d (MoE routed conv) Trainium2 Bass kernel — v2.

Strategy (v2: 1D Winograd F(2,3) along W, bf16 matmuls)
-------------------------------------------------------
Data-parallel over batch B=32 across 8 NeuronCores (4 samples/core); the
expert bank + routing params are replicated.

The 3x3 conv is computed as a direct 3-tap conv along H but Winograd
F(2,3) along W, cutting PE work 1.5x (4 transform positions replace 6
tap-products per 2 output columns).  All matmul operands are bf16
(1 cycle/row on the PE, same rate as f32r, but half the SBUF/DMA and
2-4x DVE rate for the transforms).

Per core and sample:
  1. Host pre-pads x to 58x58 and de-interleaves W into even/odd planes
     (bf16).  Device DMAs [128, 2*58*29] per c-chunk.
  2. pooled = sum(x) via DVE reduce (padding zeros are harmless);
     routing MLP on PE/ACT/DVE; softmax-normalized routing weights are
     partition-broadcast by GPSIMD.
  3. Input transform (DVE, 4 tensor_tensor adds per c-chunk):
     V0=E[t]-E[t+1], V1=D[t]+E[t+1], V2=E[t+1]-D[t], V3=D[t]-D[t+1],
     giving V[c, xi, 58, 28] bf16.
  4. Expert bank is host-transformed to the Winograd domain
     [C, E, 3dh, 4xi, O] (the weight transform is linear, so
     mix-then-transform == transform-then-mix).  Device mixing is a
     scalar_tensor_tensor chain on DVE with the (normalized) routing
     scalars -> combined[c, (dh xi o)] bf16.
  5. Conv: psum M[xi][o, 14h, 28t] accumulates 6 matmuls (2 c-chunks x
     3 dh row-shifts), N=392 rows each.  192 matmuls/sample vs 252 for
     direct conv, at 392 vs 448 rows -> 75,264 vs 112,896 PE cycles.
  6. Output transform on PSUM evac (DVE): even=M0+M1+M2, odd=M1-M2-M3,
     staged bf16 per (o-chunk, parity) and DMA'd as separate even/odd
     planes; the host re-interleaves W and converts to f32.
"""

import numpy as np
import ml_dtypes
from contextlib import ExitStack

import concourse.bass as bass
import concourse.bacc as bacc
import concourse.mybir as mybir
import concourse.tile as tile
from concourse.bass_utils import run_bass_kernel_spmd

F32 = mybir.dt.float32
BF16 = mybir.dt.bfloat16
NPBF = ml_dtypes.bfloat16
AF = mybir.ActivationFunctionType
ALU = mybir.AluOpType
AX = mybir.AxisListType

# Problem shapes (hardcoded per contract).
B, C, H, W = 32, 256, 56, 56
E, O, K = 4, 256, 3
HID = 64
NCORES = 8
BL = B // NCORES          # samples per core
CCH = C // 128            # c partition chunks
OCH = O // 128            # o partition chunks
HP = H + 2                # padded rows
NXI = 4                   # Winograd F(2,3) positions
T = W // 2                # 28 Winograd column tiles
TP = T + 1                # 29 = de-interleaved plane width (with pad col)
HBLK = 14                 # output rows per psum tile
NRB = H // HBLK           # 4 row blocks
NFREE = HBLK * T          # 392 = matmul free size
SEG = 3 * NXI * O         # 3072 = combined-weight cols per c-chunk

_CACHE = {}


def _build_program():
    nc = bacc.Bacc("TRN2", target_bir_lowering=False, debug=False)

    x_d = nc.dram_tensor("v_in", [BL, C, NXI, HP, T], BF16,
                         kind="ExternalInput").ap()
    ex_d = nc.dram_tensor("experts_w", [C, E, OCH, 3 * NXI * 128], BF16,
                          kind="ExternalInput").ap()
    # bf16 routing params: [:,0:64]=rw1t cc0, [:,64:128]=rw1t cc1,
    # [0:64,128:132]=rw2t
    rpb_d = nc.dram_tensor("rparams_bf", [128, 2 * HID + E], BF16,
                           kind="ExternalInput").ap()
    # f32 routing params: [0:64,0]=rb1, [0,1:5]=rb2
    rpf_d = nc.dram_tensor("rparams_f", [128, 1 + E], F32,
                           kind="ExternalInput").ap()
    out_d = nc.dram_tensor("out_eo", [BL, 2, O, H, T], BF16,
                           kind="ExternalOutput").ap()

    with tile.TileContext(nc) as tc, ExitStack() as ctx:
        const_pool = ctx.enter_context(tc.tile_pool(name="const", bufs=1))
        v_pool = ctx.enter_context(tc.tile_pool(name="vwin", bufs=2 * CCH))
        cmb_pool = ctx.enter_context(tc.tile_pool(name="comb", bufs=2 * CCH))
        mscr_pool = ctx.enter_context(tc.tile_pool(name="mscr", bufs=4))
        ct_pool = ctx.enter_context(tc.tile_pool(name="ct", bufs=2))
        escr_pool = ctx.enter_context(tc.tile_pool(name="escr", bufs=4))
        ostg_pool = ctx.enter_context(tc.tile_pool(name="ostg", bufs=4))
        small_pool = ctx.enter_context(tc.tile_pool(name="small", bufs=2))
        cpsum_pool = ctx.enter_context(
            tc.tile_pool(name="cpsum", bufs=7, space="PSUM"))
        mpsum_pool = ctx.enter_context(
            tc.tile_pool(name="mpsum", bufs=1, space="PSUM"))

        # ---- constants / parameters (preload once) ----
        rpb_t = const_pool.tile([128, 2 * HID + E], BF16, name="rpb")
        nc.sync.dma_start(rpb_t[:], rpb_d[:])
        rpf_t = const_pool.tile([128, 1 + E], F32, name="rpf")
        nc.sync.dma_start(rpf_t[:], rpf_d[:])
        rw1t_t = [rpb_t[:, 0:HID], rpb_t[:, HID:2 * HID]]
        rw2t_t = rpb_t[0:HID, 2 * HID:2 * HID + E]
        rb1_t = rpf_t[0:HID, 0:1]
        rb2_t = rpf_t[0:1, 1:1 + E]

        slabs = []   # [cc][e] -> [128, SEG] bf16 winograd-domain experts

        # slabs[cc][e][oc]: [128, HSEG] winograd-domain expert oc-halves
        HSEG = SEG // OCH
        for cc in range(CCH):
            slabs.append([[const_pool.tile([128, HSEG], BF16,
                                           name=f"slab{cc}e{e}o{oc}")
                           for oc in range(OCH)] for e in range(E)])

        def emit_slab_loads(oc):
            # one oc-half at a time, e-major.  The DMA pool drains roughly
            # in issue order, so these go on sync AFTER sample 0's V loads:
            # V lands first (pooling/routing path), then oc0 expert bytes
            # (sample 0's first mixing), then oc1.
            for e in range(E):
                for cc in range(CCH):
                    nc.sync.dma_start(
                        slabs[cc][e][oc][:],
                        ex_d[cc * 128:(cc + 1) * 128, e, oc])

        # per-sample state
        vt = {}       # (b, cc) -> V tile [128, NXI, HP, T] bf16
        comb = {}     # (b, cc, oc) -> combined weights [128, HSEG] bf16
        pooled_t = {}  # (b, cc) -> [128, 1] bf16 sum over h*w

        def emit_loads(b):
            # DMA only (one per Winograd plane, spread across queues); the
            # engine-side pooling runs later in emit_routing so the strict
            # ACT FIFO never stalls on these DMAs ahead of evac copies.
            for cc in range(CCH):
                v = v_pool.tile([128, NXI, HP, T], BF16, tag="v",
                                name=f"v{b}_{cc}")
                nc.sync.dma_start(
                    v.rearrange("p x h t -> p (x h t)"),
                    x_d[b, cc * 128:(cc + 1) * 128].rearrange(
                        "p x h t -> p (x h t)"))
                vt[(b, cc)] = v

        def emit_loads0_half(half):
            # sample 0's V in two waves: planes 0-1 (pooling needs plane 1)
            # land before the oc0 expert bytes, planes 2-3 right after.
            hl = NXI * HP * T // 2
            sl = slice(half * hl, (half + 1) * hl)
            for cc in range(CCH):
                if half == 0:
                    vt[(0, cc)] = v_pool.tile([128, NXI, HP, T], BF16,
                                              tag="v", name=f"v0_{cc}")
                v = vt[(0, cc)]
                nc.sync.dma_start(
                    v.rearrange("p x h t -> p (x h t)")[:, sl],
                    x_d[0, cc * 128:(cc + 1) * 128].rearrange(
                        "p x h t -> p (x h t)")[:, sl])

        def emit_pooling(b):
            # pooled(x) == sum(V1): V1 = D[t] + E[t+1] covers every unpadded
            # column of x exactly once (E[0]/D[28] are the zero pad cols).
            # ACT copy-with-accum keeps it off the DVE.
            for cc in range(CCH):
                v = vt[(b, cc)]
                pf = small_pool.tile([128, 1], F32, tag="poolf", bufs=8,
                                     name=f"poolf{b}_{cc}")
                junk = escr_pool.tile([128, HP * T], BF16, tag="junk", bufs=2,
                                      name=f"junk{b}_{cc}")
                nc.scalar.activation(junk[:],
                                     v[:, 1].rearrange("p h t -> p (h t)"),
                                     AF.Copy, accum_out=pf[:])
                pb = small_pool.tile([128, 1], BF16, tag="poolb", bufs=8,
                                     name=f"poolb{b}_{cc}")
                nc.scalar.copy(pb[:], pf[:])
                pooled_t[(b, cc)] = pb

        def emit_routing(b):
            emit_pooling(b)
            mps = mpsum_pool.tile([128, 16], F32, tag="mps", name=f"mps{b}")
            for cc in range(CCH):
                nc.tensor.matmul(mps[0:HID, 0:1], rw1t_t[cc],
                                 pooled_t[(b, cc)][:],
                                 start=(cc == 0), stop=(cc == CCH - 1))
            h_sb = small_pool.tile([HID, 1], BF16, tag="h", name=f"h{b}")
            nc.scalar.activation(h_sb[:], mps[0:HID, 0:1], AF.Relu,
                                 bias=rb1_t[:])
            nc.tensor.matmul(mps[0:1, 4:4 + E], h_sb[:], rw2t_t[:],
                             start=True, stop=True)
            ze = small_pool.tile([1, E], F32, tag="ze", name=f"ze{b}")
            nc.vector.tensor_add(ze[:], mps[0:1, 4:4 + E], rb2_t[:])
            es = small_pool.tile([1, E], F32, tag="es", name=f"es{b}")
            nc.scalar.activation(es[:], ze[:], AF.Exp)
            ssum = small_pool.tile([1, 1], F32, tag="ssum", name=f"ss{b}")
            nc.vector.reduce_sum(out=ssum[:], in_=es[:], axis=AX.X)
            rec = small_pool.tile([1, 1], F32, tag="rec", name=f"rec{b}")
            nc.vector.reciprocal(rec[:], ssum[:])
            esn = small_pool.tile([1, E], F32, tag="esn", name=f"esn{b}")
            nc.vector.tensor_scalar_mul(esn[:], es[:], rec[0:1, 0:1])
            rbc = small_pool.tile([128, E], F32, tag="rbc", name=f"rbc{b}")
            nc.gpsimd.partition_broadcast(rbc[:], esn[0:1, 0:E])
            return rbc

        def emit_mixing(b, rbc, oc):
            # tensor_scalar_mul hits the DVE bf16 fast path (~3.5x faster
            # than scalar_tensor_tensor); tree-add the 4 scaled slabs.
            for cc in range(CCH):
                slab = slabs[cc]
                m = [mscr_pool.tile([128, HSEG], BF16, tag="mscr",
                                    name=f"ms{b}_{cc}_{oc}_{e}")
                     for e in range(E)]
                # chained adds so each step can run as soon as its slab's
                # DMA lands (e-major arrival order)
                nc.vector.tensor_scalar_mul(m[0][:], slab[0][oc][:],
                                            rbc[:, 0:1])
                nc.vector.tensor_scalar_mul(m[1][:], slab[1][oc][:],
                                            rbc[:, 1:2])
                nc.vector.tensor_add(m[0][:], m[0][:], m[1][:])
                nc.vector.tensor_scalar_mul(m[2][:], slab[2][oc][:],
                                            rbc[:, 2:3])
                nc.vector.tensor_add(m[0][:], m[0][:], m[2][:])
                # last term on ACT: relieves the ~86%-busy DVE
                nc.scalar.mul(m[3][:], slab[3][oc][:], rbc[:, 3:4])
                cmb = cmb_pool.tile([128, HSEG], BF16, tag="cmb",
                                    name=f"cmb{b}_{cc}_{oc}")
                nc.vector.tensor_add(cmb[:], m[0][:], m[3][:])
                comb[(b, cc, oc)] = cmb

        def emit_conv_ochunk(b, oc):
            oeven = ostg_pool.tile([128, H * T], BF16, tag="ostg",
                                   name=f"oe{b}_{oc}")
            oodd = ostg_pool.tile([128, H * T], BF16, tag="ostg",
                                  name=f"oo{b}_{oc}")
            ct = None
            for rb in range(NRB):
                # ACT evacuates psum into a bf16 staging tile (one psum read
                # per op); DVE then runs the output transform as few, fat,
                # all-SBUF bf16 ops over rb-pairs.
                if rb % 2 == 0:
                    ct = ct_pool.tile([128, NXI, 2 * NFREE], BF16, tag="ct",
                                      name=f"ct{b}_{oc}_{rb // 2}")
                pt = [cpsum_pool.tile([128, NFREE], F32, tag="cps",
                                      name=f"cp{b}_{oc}_{rb}_{xi}")
                      for xi in range(NXI)]
                i = 0
                for cc in range(CCH):
                    cmb = comb[(b, cc, oc)]
                    v = vt[(b, cc)]
                    for dh in range(3):
                        for xi in range(NXI):
                            lo = (dh * NXI + xi) * 128
                            rhs = v[:, xi, rb * HBLK + dh: rb * HBLK + dh + HBLK,
                                    0:T]
                            nc.tensor.matmul(pt[xi][:], cmb[:, lo:lo + 128],
                                             rhs, start=(i < NXI),
                                             stop=(i >= (2 * 3 - 1) * NXI))
                        i += NXI
                hs = slice((rb % 2) * NFREE, (rb % 2 + 1) * NFREE)
                for xi in range(NXI):
                    nc.scalar.copy(ct[:, xi, hs], pt[xi][:])
                if rb % 2 == 1:
                    sl = slice((rb - 1) * NFREE, (rb + 1) * NFREE)
                    t1 = escr_pool.tile([128, 2 * NFREE], BF16, tag="escr",
                                        name=f"t1{b}_{oc}_{rb}")
                    nc.vector.tensor_add(t1[:], ct[:, 0], ct[:, 1])
                    nc.vector.tensor_add(oeven[:, sl], t1[:], ct[:, 2])
                    t2 = escr_pool.tile([128, 2 * NFREE], BF16, tag="escr",
                                        name=f"t2{b}_{oc}_{rb}")
                    nc.vector.tensor_sub(t2[:], ct[:, 1], ct[:, 2])
                    nc.vector.tensor_sub(oodd[:, sl], t2[:], ct[:, 3])
                    if b == BL - 1 and oc == OCH - 1:
                        # stream the final chunk per rb-pair: shortens the
                        # drain after the last matmul
                        ovp = out_d[b, :, oc * 128:(oc + 1) * 128,
                                    (rb - 1) * HBLK:(rb + 1) * HBLK].rearrange(
                            "a p h t -> a p (h t)")
                        nc.gpsimd.dma_start(ovp[0], oeven[:, sl])
                        nc.gpsimd.dma_start(ovp[1], oodd[:, sl])
            if b == BL - 1 and oc == OCH - 1:
                return
            # out DMAs issue from the idle GPSIMD queue (sync stays free
            # for input loads)
            ov = out_d[b, :, oc * 128:(oc + 1) * 128].rearrange(
                "a p h t -> a p (h t)")
            nc.gpsimd.dma_start(ov[0], oeven[:])
            nc.gpsimd.dma_start(ov[1], oodd[:])

        # ---- emission: software-pipelined across samples ----
        emit_loads0_half(0)
        emit_slab_loads(0)
        emit_loads0_half(1)
        emit_slab_loads(1)
        route = {0: emit_routing(0)}
        emit_mixing(0, route[0], 0)
        for b in range(BL):
            emit_mixing(b, route[b], 1)
            emit_conv_ochunk(b, 0)
            if b + 1 < BL:
                emit_loads(b + 1)
                route[b + 1] = emit_routing(b + 1)
                emit_mixing(b + 1, route[b + 1], 0)
            emit_conv_ochunk(b, 1)

    nc.compile()
    return nc


def _prep_inputs(x, experts, rw1, rb1, rw2, rb2):
    x = np.asarray(x, dtype=np.float32)
    xp = np.pad(x, ((0, 0), (0, 0), (1, 1), (1, 1)))
    # Winograd F(2,3) input transform along W (linear data prep):
    # [B, C, 4xi, 58, 28]
    ev = xp[:, :, :, 0::2]
    dv = xp[:, :, :, 1::2]
    v_in = np.ascontiguousarray(np.stack([
        ev[:, :, :, 0:T] - ev[:, :, :, 1:TP],
        dv[:, :, :, 0:T] + ev[:, :, :, 1:TP],
        ev[:, :, :, 1:TP] - dv[:, :, :, 0:T],
        dv[:, :, :, 0:T] - dv[:, :, :, 1:TP],
    ], axis=2)).astype(NPBF)
    g = np.asarray(experts, dtype=np.float32)        # [E, O, C, 3, 3]
    w0, w1, w2 = g[..., 0], g[..., 1], g[..., 2]     # [E, O, C, 3dh]
    wt = np.stack([w0, (w0 + w1 + w2) * 0.5, (w0 - w1 + w2) * 0.5, w2],
                  axis=-1)                           # [E, O, C, 3dh, 4xi]
    # [C, E, 3, 4, O] -> oc-major halves [C, E, 2oc, (3 4 128)]
    ex_t = np.transpose(wt, (2, 0, 3, 4, 1)).reshape(C, E, 3 * NXI, OCH, 128)
    ex_t = np.ascontiguousarray(
        np.transpose(ex_t, (0, 1, 3, 2, 4))).astype(NPBF)
    ex_t = ex_t.reshape(C, E, OCH, 3 * NXI * 128)
    rw1t = (np.asarray(rw1, dtype=np.float32) / float(H * W)).T  # [C, HID]
    rw2t = np.asarray(rw2, dtype=np.float32).T                   # [HID, E]
    rpb = np.zeros((128, 2 * HID + E), np.float32)
    rpb[:, 0:HID] = rw1t[0:128]
    rpb[:, HID:2 * HID] = rw1t[128:256]
    rpb[0:HID, 2 * HID:2 * HID + E] = rw2t
    rpb = rpb.astype(NPBF)
    rpf = np.zeros((128, 1 + E), np.float32)
    rpf[0:HID, 0] = np.asarray(rb1, dtype=np.float32)
    rpf[0, 1:1 + E] = np.asarray(rb2, dtype=np.float32)
    in_maps = []
    for i in range(NCORES):
        in_maps.append({
            "v_in": np.ascontiguousarray(v_in[i * BL:(i + 1) * BL]),
            "experts_w": ex_t,
            "rparams_bf": rpb,
            "rparams_f": rpf,
        })
    return in_maps


def run(inputs, trace=False, **trace_kwargs):
    """Build (cached), run on 8 cores, return (full_out, BassKernelResults)."""
    trace_kwargs.pop("use_f32r", None)
    if "prog" not in _CACHE:
        _CACHE["prog"] = _build_program()
    nc = _CACHE["prog"]
    in_maps = _prep_inputs(**inputs)
    res = run_bass_kernel_spmd(nc, in_maps, list(range(NCORES)),
                               trace=trace, **trace_kwargs)
    out = np.empty((B, O, H, W), dtype=np.float32)
    for i in range(NCORES):
        oeo = np.asarray(res.results[i]["out_eo"], dtype=np.float32)
        out[i * BL:(i + 1) * BL, :, :, 0::2] = oeo[:, 0]
        out[i * BL:(i + 1) * BL, :, :, 1::2] = oeo[:, 1]
    return out, res


def kernel(x, experts, rw1, rb1, rw2, rb2):
    out, _ = run(dict(x=x, experts=experts, rw1=rw1, rb1=rb1, rw2=rw2,
                      rb2=rb2))
    return out
